# revision 29
# baseline (speedup 1.0000x reference)
"""LSG (local-sparse-global) block-local self-attention for Trainium2.

Problem: n=2, h=16, t=4096, d=64, block=128. Each query block attends to a
3-block local key window (1-block halo each side) plus a global BOS token
slot; the BOS query (position 0) attends to everything.

Strategy (8 NeuronCores, batch*head = 32 sharded 4 per core, as 2 "bh
pairs" per core):

  - QK^T: the two bh of a pair ride the two PE row groups (contraction
    d=64 each) concurrently: lhsT = kT(bh0) on partitions 0-63 and
    kT(bh1) on 64-127, each against its own 384-wide query union, into
    separate PSUM banks of one score tile. Full-array utilization, no q
    duplication, and the shared tile keeps the Tile scheduler from
    splitting the pair.
  - softmax has no running max: p = exp(s/8). Scores/8 are ~N(0,1) so
    plain exp stays in range and constant bias cancels after the host
    normalization. Exp alternates per key block between ACT (exact
    spline Exp) and DVE using a single-pass Schraudolph bit-trick:
    i16 = round(s*23.083 + b) written via an int16 bitcast IS the bf16
    encoding of exp(s/8) (+-3% sawtooth, mostly cancelling after
    normalization; ~1e-2 absmax vs the 2e-2 gate).
  - PV runs transposed so M=128 (full array): out[q, 0:65] accumulates
    pt_j^T @ [v_j | 1] with pt (keys x queries) stationary and the
    65-wide v-augmented block moving. Accumulation rides per-element
    has_written PSUM semantics in per-4-query-block "generation" banks:
    one start=True on the bank's first write per generation, everything
    else accumulates or first-writes naturally.
  - PV trails the scores by 2 key blocks so the PE never waits on the
    exp that was just issued; gen copies split across ACT/DVE; outputs
    (64 dims + softmax sum) leave as bf16 via the gpsimd DMA queue.
  - Host divides by sums, adds the BOS-token key slot for query blocks
    >= 2 (blocks 0/1 already have key 0 in-window, matching the
    reference's global-slot semantics), and computes the single BOS
    query row (~0.5% of FLOPs).
"""

import sys

import numpy as np
import ml_dtypes

try:  # concourse (bass) ships in the trn_rl repo, not on the default path
    import concourse.bass  # noqa: F401
except ImportError:
    for _p in ("/opt/trn_rl_repo", "/root/.axon_site/_ro/trn_rl_repo"):
        if _p not in sys.path:
            sys.path.insert(0, _p)

N, H, T, D = 2, 16, 4096, 64
BLOCK = 128
NB = T // BLOCK            # 32 key/query blocks
BH = N * H                 # 32 batch*head pairs
NCORES = 8
BH_PER_CORE = BH // NCORES  # 4
NPAIR = BH_PER_CORE // 2    # 2 bh-pairs per core
SCALE = 1.0 / 8.0           # 1/sqrt(64)
VW = D + 1                  # 65: v columns + ones column
GSTRIDE = 66                # psum column stride per query-block group (8B align)

# exp split: key blocks in DVE_JS run the DVE Schraudolph exp, others ACT
DVE_JS = frozenset(range(0, 32, 2))
SCH_C = 0.02
SCH_A = float(128.0 * np.log2(np.e) * SCALE)     # 23.083
SCH_B = float(16256.0 - SCH_C * 128.0)

_BF16 = ml_dtypes.bfloat16

_CACHE = {}


def _build_bass():
    import concourse.bacc as bacc
    import concourse.mybir as mybir
    import concourse.tile as tile

    bf16 = mybir.dt.bfloat16
    i16 = mybir.dt.int16
    f32 = mybir.dt.float32

    nc = bacc.Bacc(None, target_bir_lowering=False)
    # qt/kt: [pair, 128, T]: partitions 0-63 = bh even (d-major transpose),
    # 64-127 = bh odd.
    qt = nc.declare_dram_parameter("qt", [NPAIR, 128, T], bf16, isOutput=False)
    kt = nc.declare_dram_parameter("kt", [NPAIR, 128, T], bf16, isOutput=False)
    # va: [pair, bh_slot, 128, NB*65]: per key block j, cols 65j..65j+64 hold
    # [v[128j + p, :], 1.0] on partition p.
    va = nc.declare_dram_parameter(
        "va", [NPAIR, 2, 128, NB * VW], bf16, isOutput=False
    )
    # out: [pair, 8, 128, 520]: tile t holds query blocks 4t..4t+3 for both
    # bh slots as [gen2, qb2, s2, 65] col groups; partition dim = query.
    out = nc.declare_dram_parameter(
        "out", [NPAIR, 8, 128, 2 * 4 * VW], bf16, isOutput=True
    )

    AH = 17 * BLOCK     # 2176: B-half tile width (blocks 15-31)

    with tile.TileContext(nc) as tc:
        with (
            tc.tile_pool(name="cst", bufs=1) as cst,
            tc.tile_pool(name="sbq", bufs=1) as sbq,
            tc.tile_pool(name="sbk", bufs=1) as sbk,
            tc.tile_pool(name="sbv", bufs=1) as sbv,
            tc.tile_pool(name="sbp", bufs=6) as sbp,
            tc.tile_pool(name="sbo", bufs=2) as sbo,
            tc.tile_pool(name="psS", bufs=3, space="PSUM") as psS,
            tc.tile_pool(name="psG", bufs=2, space="PSUM") as psG,
        ):
            bias_tile = cst.tile([128, 1], f32, tag="bias")
            nc.vector.memset(bias_tile, 0.0)
            # Touch Exp from ACT once: loads the exp table set during warmup
            # and keeps later Exp ops from each carrying a cross-engine wait.
            warm = cst.tile([128, 1], f32, tag="warm")
            nc.scalar.activation(
                out=warm,
                in_=bias_tile,
                func=mybir.ActivationFunctionType.Exp,
                bias=0.0,
                scale=1.0,
            )
            # Short PE warmup: the lead-in chunks land quickly, so just keep
            # the PE pipeline alive until they do.
            wsrc = cst.tile([64, 512], bf16, tag="wsrc")
            nc.vector.memset(wsrc, 0.0)
            wps = psS.tile([128, 1024], f32, tag="spair", name="warm_ps")
            for _ in range(10):
                nc.tensor.matmul(
                    out=wps[:, 0:512],
                    lhsT=wsrc[:, 0:128],
                    rhs=wsrc[:, :],
                    start=True,
                    stop=True,
                )

            # ---- input loads (sync engine queue). Lead-in chunks measured
            # net-neutral-to-negative (more DMA descriptors during the
            # bandwidth-bound input era); disabled.
            MINIS = False
            tiles = []
            for pair in range(NPAIR):
                d = {}
                if MINIS and pair == 0:
                    d["q0"] = sbq.tile([128, 5 * BLOCK], bf16, tag="q0", name="q0")
                    d["k0"] = sbk.tile([128, 4 * BLOCK], bf16, tag="k0", name="k0")
                    nc.sync.dma_start(out=d["q0"], in_=qt[0, :, 0 : 5 * BLOCK])
                    nc.sync.dma_start(out=d["k0"], in_=kt[0, :, 0 : 4 * BLOCK])
                    d["v0"] = [None, None]
                    for s in range(2):
                        t0 = sbv.tile([128, 4 * VW], bf16, tag=f"v0{s}", name=f"v0{s}")
                        nc.sync.dma_start(out=t0, in_=va[0, s, :, 0 : 4 * VW])
                        d["v0"][s] = t0
                    # A covers blocks 3-16 (q) / 4-15 (k, va) for j in 4..15
                    d["qA"] = sbq.tile(
                        [128, 14 * BLOCK], bf16, tag=f"qA{pair}", name=f"qA{pair}"
                    )
                    d["qA_base"] = 3 * BLOCK
                    d["kA"] = sbk.tile(
                        [128, 12 * BLOCK], bf16, tag=f"kA{pair}", name=f"kA{pair}"
                    )
                    d["kA_base"] = 4
                    nc.sync.dma_start(
                        out=d["qA"], in_=qt[0, :, 3 * BLOCK : 17 * BLOCK]
                    )
                    nc.sync.dma_start(
                        out=d["kA"], in_=kt[0, :, 4 * BLOCK : 16 * BLOCK]
                    )
                    d["vA"] = [None, None]
                    d["vA_base"] = 4
                    for s in range(2):
                        tA = sbv.tile(
                            [128, 12 * VW], bf16, tag=f"vA{pair}{s}", name=f"vA{pair}{s}"
                        )
                        nc.sync.dma_start(out=tA, in_=va[0, s, :, 4 * VW : 16 * VW])
                        d["vA"][s] = tA
                else:
                    d["qA"] = sbq.tile(
                        [128, AH], bf16, tag=f"qA{pair}", name=f"qA{pair}"
                    )
                    d["qA_base"] = 0
                    d["kA"] = sbk.tile(
                        [128, 16 * BLOCK], bf16, tag=f"kA{pair}", name=f"kA{pair}"
                    )
                    d["kA_base"] = 0
                    nc.sync.dma_start(out=d["qA"], in_=qt[pair, :, 0:AH])
                    nc.sync.dma_start(out=d["kA"], in_=kt[pair, :, 0 : 16 * BLOCK])
                    d["vA"] = [None, None]
                    d["vA_base"] = 0
                    for s in range(2):
                        tA = sbv.tile(
                            [128, 16 * VW], bf16, tag=f"vA{pair}{s}", name=f"vA{pair}{s}"
                        )
                        nc.sync.dma_start(out=tA, in_=va[pair, s, :, 0 : 16 * VW])
                        d["vA"][s] = tA
                d["qB"] = sbq.tile([128, AH], bf16, tag=f"qB{pair}", name=f"qB{pair}")
                d["kB"] = sbk.tile(
                    [128, 16 * BLOCK], bf16, tag=f"kB{pair}", name=f"kB{pair}"
                )
                nc.sync.dma_start(out=d["qB"], in_=qt[pair, :, T - AH : T])
                nc.sync.dma_start(out=d["kB"], in_=kt[pair, :, 16 * BLOCK : T])
                d["vB"] = [None, None]
                for s in range(2):
                    tB = sbv.tile(
                        [128, 16 * VW], bf16, tag=f"vB{pair}{s}", name=f"vB{pair}{s}"
                    )
                    nc.sync.dma_start(out=tB, in_=va[pair, s, :, 16 * VW : NB * VW])
                    d["vB"][s] = tB
                tiles.append(d)

            for pair in range(NPAIR):
                d = tiles[pair]
                pts = {}     # j -> pt tile [128, 768]
                gens = {}    # (s, g) -> psum generation tile
                obts = {}    # (s, t) -> output staging tile

                def qsel(j):
                    if "q0" in d and j <= 3:
                        return d["q0"], 0
                    if j <= 15:
                        return d["qA"], d["qA_base"]
                    return d["qB"], T - AH

                def ksel(j):
                    if "k0" in d and j <= 3:
                        return d["k0"], 0
                    if j <= 15:
                        return d["kA"], d["kA_base"]
                    return d["kB"], 16

                def vsel(j, s):
                    if "v0" in d and j <= 3:
                        return d["v0"][s], 0
                    if j <= 15:
                        return d["vA"][s], d["vA_base"]
                    return d["vB"][s], 16

                def qk(j):
                    """scores^T for key block j of both bh (concurrent)."""
                    sP = psS.tile([128, 1024], f32, tag="spair", name=f"sP{pair}_{j}")
                    # query union = blocks j-1..j+1 clipped; window pos w
                    # covers query block j-1+w at psum cols 128w (+512 bh1)
                    wlo = 1 if j == 0 else 0
                    whi = 2 if j == NB - 1 else 3
                    kta, kbase = ksel(j)
                    qta, qbase = qsel(j)
                    kcol = (j - kbase) * BLOCK
                    qlo = (j - 1 + wlo) * BLOCK - qbase
                    n = (whi - wlo) * BLOCK
                    for s in range(2):
                        p0, p1 = (0, 64) if s == 0 else (64, 128)
                        nc.tensor.matmul(
                            out=sP[:, 512 * s + wlo * BLOCK : 512 * s + wlo * BLOCK + n],
                            lhsT=kta[p0:p1, kcol : kcol + BLOCK],
                            rhs=qta[p0:p1, qlo : qlo + n],
                            start=True,
                            stop=True,
                        )
                    # exp into SBUF bf16 pt tile: [0:384]=bh0, [384:768]=bh1
                    ptj = sbp.tile([128, 768], bf16, tag="pt", name=f"pt{pair}_{j}")
                    s_view = sP.rearrange("p (b w) -> p b w", b=2)[
                        :, :, wlo * BLOCK : whi * BLOCK
                    ]
                    p_view = ptj.rearrange("p (b w) -> p b w", b=2)[
                        :, :, wlo * BLOCK : whi * BLOCK
                    ]
                    if j in DVE_JS:
                        nc.vector.tensor_scalar(
                            out=p_view.bitcast(i16),
                            in0=s_view,
                            scalar1=SCH_A,
                            scalar2=SCH_B,
                            op0=mybir.AluOpType.mult,
                            op1=mybir.AluOpType.add,
                        )
                    else:
                        nc.scalar.activation(
                            out=p_view,
                            in_=s_view,
                            func=mybir.ActivationFunctionType.Exp,
                            bias=bias_tile[:, :],
                            scale=SCALE,
                        )
                    pts[j] = ptj

                def pvq(qb):
                    """One query block's full accumulation: both bh, all 3
                    window keys, back-to-back into the generation bank that
                    holds query blocks (2b, 2b+1) x (bh0, bh1). Generations
                    never overlap, so 2 PSUM banks suffice and the score
                    ring gets depth 3."""
                    b = qb // 2
                    if b not in gens:
                        gens[b] = psG.tile(
                            [128, 512], f32, tag="g", name=f"gen{pair}_{b}"
                        )
                    gt = gens[b]
                    first_of_bank = qb == 2 * b
                    klo, khi = max(0, qb - 1), min(NB - 1, qb + 1)
                    for s in range(2):
                        o = 2 * GSTRIDE * (qb % 2) + GSTRIDE * s
                        for j in range(klo, khi + 1):
                            ptj = pts[j]
                            vaa, vbase = vsel(j, s)
                            w = qb - (j - 1)  # window position 0..2
                            nc.tensor.matmul(
                                out=gt[:, o : o + VW],
                                lhsT=ptj[:, 384 * s + w * BLOCK : 384 * s + w * BLOCK + BLOCK],
                                rhs=vaa[:, (j - vbase) * VW : (j - vbase) * VW + VW],
                                start=(first_of_bank and s == 0 and j == klo),
                                stop=(qb == 2 * b + 1 and s == 1 and j == khi),
                                skip_group_check=True,
                            )

                def close_gen(b):
                    """Copy finished generation bank to staging; DMA per 2.
                    Copy engine alternates so neither ACT nor DVE eats the
                    full copy load on top of its exp share."""
                    gt = gens.pop(b)
                    t = b // 2
                    if t not in obts:
                        obts[t] = sbo.tile(
                            [128, 2 * 4 * VW], bf16, tag="ob", name=f"ob{pair}_{t}"
                        )
                    obt = obts[t]
                    dst = obt.rearrange("p (b w) -> p b w", w=VW)[
                        :, 4 * (b % 2) : 4 * (b % 2) + 4, :
                    ]
                    src = gt[:, 0 : 4 * GSTRIDE].rearrange(
                        "p (b w) -> p b w", w=GSTRIDE
                    )[:, :, 0:VW]
                    if b % 2 == 0:
                        nc.scalar.copy(out=dst, in_=src)
                    else:
                        nc.vector.tensor_copy(out=dst, in_=src)
                    if b % 2 == 1:
                        obts.pop(t)
                        nc.gpsimd.dma_start(out=out[pair, t], in_=obt)

                # PV trails the scores by 3+ key blocks (group qb needs
                # pt_{qb+1}), so the PE never waits on a fresh exp. Two
                # groups are emitted per two score steps: each QK<->PV
                # transition on the PE costs ~100ns extra on the first PV
                # matmul, so batching halves that overhead.
                for j in range(NB):
                    qk(j)
                    if j % 2 == 1:
                        for qq in (j - 4, j - 3):
                            if qq >= 0:
                                pvq(qq)
                                if qq % 2 == 1:
                                    close_gen(qq // 2)
                for qq in (NB - 3, NB - 2, NB - 1):
                    pvq(qq)
                    if qq % 2 == 1:
                        close_gen(qq // 2)
                pts.clear()
    nc.compile()
    return nc


def _host_tensors(q, k, v):
    """Build device input arrays from [BH, T, D] fp32 q/k/v.

    qt/kt [BH//2, 128, T]: d-major transposes, bh even on partitions 0-63,
        bh odd on 64-127.
    va [BH//2, 2, 128, NB*65]: per key block j, cols 65j..65j+64 hold
        [v[128j + p, :], 1.0] on partition p.
    """
    qtT = np.ascontiguousarray(q.transpose(0, 2, 1)).astype(_BF16)  # [BH, 64, T]
    ktT = np.ascontiguousarray(k.transpose(0, 2, 1)).astype(_BF16)
    qt = qtT.reshape(BH // 2, 128, T)
    kt = ktT.reshape(BH // 2, 128, T)

    va = np.empty((BH, 128, NB, VW), dtype=_BF16)
    va[:, :, :, :D] = v.reshape(BH, NB, BLOCK, D).transpose(0, 2, 1, 3)
    va[:, :, :, D] = np.float32(1.0)
    va = va.reshape(BH // 2, 2, 128, NB * VW)
    return qt, kt, va


def _in_maps(qt, kt, va):
    maps = []
    for c in range(NCORES):
        s = slice(c * NPAIR, (c + 1) * NPAIR)
        maps.append({"qt": qt[s], "kt": kt[s], "va": va[s]})
    return maps


def _epilogue(outT, q, k, v, mask):
    """outT: [BH//2, 8, 128, 520] bf16 device result -> [N,H,T,D] f32."""
    # 520 = [gen2, qb2, s2, 65]; qb_global = t*4 + gen*2 + qb; bh = 2*pair+s
    o = outT.astype(np.float32).reshape(BH // 2, 8, 128, 2, 2, 2, VW)
    o = o.transpose(0, 5, 1, 3, 4, 2, 6).reshape(BH, NB * BLOCK, VW)
    sums = np.ascontiguousarray(o[:, :, D])      # [BH, T]
    o = np.ascontiguousarray(o[:, :, 0:D])       # [BH, T, D]

    # BOS-token key slot for query blocks >= 2 (blocks 0/1 already have key 0
    # inside their local window, which equals the reference's global slot).
    k0 = k[:, 0, :]
    v0 = v[:, 0, :]
    qs = q[:, 2 * BLOCK :, :]
    pk = np.exp(np.einsum("bqd,bd->bq", qs, k0) * SCALE)
    o[:, 2 * BLOCK :, :] += pk[:, :, None] * v0[:, None, :]
    sums[:, 2 * BLOCK :] += pk

    o /= sums[:, :, None]

    # BOS query row: full attention of query 0 over all T keys.
    mrow = np.repeat(mask[:, 0, 0, :], H, axis=0)  # [BH, T]
    s0 = np.einsum("bd,btd->bt", q[:, 0, :], k) * SCALE + mrow
    s0 -= s0.max(axis=1, keepdims=True)
    p0 = np.exp(s0)
    p0 /= p0.sum(axis=1, keepdims=True)
    o[:, 0, :] = np.einsum("bt,btd->bd", p0, v)

    return o.reshape(N, H, T, D).astype(np.float32)


def kernel(query_layer, key_layer, value_layer, attention_mask):
    from concourse.bass_utils import run_bass_kernel_spmd

    q = np.asarray(query_layer, dtype=np.float32).reshape(BH, T, D)
    k = np.asarray(key_layer, dtype=np.float32).reshape(BH, T, D)
    v = np.asarray(value_layer, dtype=np.float32).reshape(BH, T, D)
    mask = np.asarray(attention_mask, dtype=np.float32)  # [N,1,1,T]

    qt, kt, va = _host_tensors(q, k, v)

    if "nc" not in _CACHE:
        _CACHE["nc"] = _build_bass()
    nc = _CACHE["nc"]

    res = run_bass_kernel_spmd(nc, _in_maps(qt, kt, va), core_ids=list(range(NCORES)))
    outT = np.concatenate([np.asarray(r["out"]) for r in res.results], axis=0)
    return _epilogue(outT, q, k, v, mask)


# revision 33
# speedup vs baseline: 1.0798x; 1.0798x over previous
"""LSG (local-sparse-global) block-local self-attention for Trainium2.

Problem: n=2, h=16, t=4096, d=64, block=128. Each query block attends to a
3-block local key window (1-block halo each side) plus a global BOS token
slot; the BOS query (position 0) attends to everything.

Strategy (8 NeuronCores, batch*head = 32 sharded 4 per core, as 2 "bh
pairs" per core):

  - QK^T: the two bh of a pair ride the two PE row groups (contraction
    d=64 each) concurrently: lhsT = kT(bh0) on partitions 0-63 and
    kT(bh1) on 64-127, each against its own 384-wide query union, into
    separate PSUM banks of one score tile. Full-array utilization, no q
    duplication, and the shared tile keeps the Tile scheduler from
    splitting the pair.
  - softmax has no running max: p = exp(s/8). Scores/8 are ~N(0,1) so
    plain exp stays in range and constant bias cancels after the host
    normalization. Exp alternates per key block between ACT (exact
    spline Exp) and DVE using a single-pass Schraudolph bit-trick:
    i16 = round(s*23.083 + b) written via an int16 bitcast IS the bf16
    encoding of exp(s/8) (+-3% sawtooth, mostly cancelling after
    normalization; ~1e-2 absmax vs the 2e-2 gate).
  - PV runs transposed so M=128 (full array): out[q, 0:65] accumulates
    pt_j^T @ [v_j | 1] with pt (keys x queries) stationary and the
    65-wide v-augmented block moving. Accumulation rides per-element
    has_written PSUM semantics in per-4-query-block "generation" banks:
    one start=True on the bank's first write per generation, everything
    else accumulates or first-writes naturally.
  - PV trails the scores by 2 key blocks so the PE never waits on the
    exp that was just issued; gen copies split across ACT/DVE; outputs
    (64 dims + softmax sum) leave as bf16 via the gpsimd DMA queue.
  - Host divides by sums, adds the BOS-token key slot for query blocks
    >= 2 (blocks 0/1 already have key 0 in-window, matching the
    reference's global-slot semantics), and computes the single BOS
    query row (~0.5% of FLOPs).
"""

import sys

import numpy as np
import ml_dtypes

try:  # concourse (bass) ships in the trn_rl repo, not on the default path
    import concourse.bass  # noqa: F401
except ImportError:
    for _p in ("/opt/trn_rl_repo", "/root/.axon_site/_ro/trn_rl_repo"):
        if _p not in sys.path:
            sys.path.insert(0, _p)

N, H, T, D = 2, 16, 4096, 64
BLOCK = 128
NB = T // BLOCK            # 32 key/query blocks
BH = N * H                 # 32 batch*head pairs
NCORES = 8
BH_PER_CORE = BH // NCORES  # 4
NPAIR = BH_PER_CORE // 2    # 2 bh-pairs per core
SCALE = 1.0 / 8.0           # 1/sqrt(64)
VW = D + 1                  # 65: v columns + ones column
GSTRIDE = 66                # psum column stride per query-block group (8B align)

# exp split: key blocks in DVE_JS run the DVE Schraudolph exp, others ACT
DVE_JS = frozenset(range(0, 32, 2))
SCH_C = 0.02
SCH_A = float(128.0 * np.log2(np.e) * SCALE)     # 23.083
SCH_B = float(16256.0 - SCH_C * 128.0)

_BF16 = ml_dtypes.bfloat16

_CACHE = {}


def _build_bass():
    import concourse.bacc as bacc
    import concourse.mybir as mybir
    import concourse.tile as tile

    bf16 = mybir.dt.bfloat16
    i16 = mybir.dt.int16
    f32 = mybir.dt.float32

    nc = bacc.Bacc(None, target_bir_lowering=False)
    # qt/kt: [pair, 128, T]: partitions 0-63 = bh even (d-major transpose),
    # 64-127 = bh odd.
    qt = nc.declare_dram_parameter("qt", [NPAIR, 128, T], bf16, isOutput=False)
    kt = nc.declare_dram_parameter("kt", [NPAIR, 128, T], bf16, isOutput=False)
    # va: [pair, bh_slot, 128, NB*65]: per key block j, cols 65j..65j+64 hold
    # [v[128j + p, :], 1.0] on partition p.
    va = nc.declare_dram_parameter(
        "va", [NPAIR, 2, 128, NB * VW], bf16, isOutput=False
    )
    # out: [pair, 8, 128, 520]: tile t holds query blocks 4t..4t+3 for both
    # bh slots as [gen2, qb2, s2, 65] col groups; partition dim = query.
    out = nc.declare_dram_parameter(
        "out", [NPAIR, 8, 128, 2 * 4 * VW], bf16, isOutput=True
    )

    AH = 17 * BLOCK     # 2176: B-half tile width (blocks 15-31)

    with tile.TileContext(nc) as tc:
        with (
            tc.tile_pool(name="cst", bufs=1) as cst,
            tc.tile_pool(name="sbq", bufs=1) as sbq,
            tc.tile_pool(name="sbk", bufs=1) as sbk,
            tc.tile_pool(name="sbv", bufs=1) as sbv,
            tc.tile_pool(name="sbp", bufs=6) as sbp,
            tc.tile_pool(name="sbo", bufs=2) as sbo,
            tc.tile_pool(name="psS", bufs=3, space="PSUM") as psS,
            tc.tile_pool(name="psG", bufs=2, space="PSUM") as psG,
        ):
            bias_tile = cst.tile([128, 1], f32, tag="bias")
            nc.vector.memset(bias_tile, 0.0)
            # Touch Exp from ACT once: loads the exp table set during warmup
            # and keeps later Exp ops from each carrying a cross-engine wait.
            warm = cst.tile([128, 1], f32, tag="warm")
            nc.scalar.activation(
                out=warm,
                in_=bias_tile,
                func=mybir.ActivationFunctionType.Exp,
                bias=0.0,
                scale=1.0,
            )
            # Short PE warmup: the lead-in chunks land quickly, so just keep
            # the PE pipeline alive until they do.
            wsrc = cst.tile([64, 512], bf16, tag="wsrc")
            nc.vector.memset(wsrc, 0.0)
            wps = psS.tile([128, 1024], f32, tag="spair", name="warm_ps")
            for _ in range(10):
                nc.tensor.matmul(
                    out=wps[:, 0:512],
                    lhsT=wsrc[:, 0:128],
                    rhs=wsrc[:, :],
                    start=True,
                    stop=True,
                )

            # ---- input loads (sync engine queue). Lead-in chunks measured
            # net-neutral-to-negative (more DMA descriptors during the
            # bandwidth-bound input era); disabled.
            MINIS = False
            tiles = []
            for pair in range(NPAIR):
                d = {}
                if MINIS and pair == 0:
                    d["q0"] = sbq.tile([128, 5 * BLOCK], bf16, tag="q0", name="q0")
                    d["k0"] = sbk.tile([128, 4 * BLOCK], bf16, tag="k0", name="k0")
                    nc.sync.dma_start(out=d["q0"], in_=qt[0, :, 0 : 5 * BLOCK])
                    nc.sync.dma_start(out=d["k0"], in_=kt[0, :, 0 : 4 * BLOCK])
                    d["v0"] = [None, None]
                    for s in range(2):
                        t0 = sbv.tile([128, 4 * VW], bf16, tag=f"v0{s}", name=f"v0{s}")
                        nc.sync.dma_start(out=t0, in_=va[0, s, :, 0 : 4 * VW])
                        d["v0"][s] = t0
                    # A covers blocks 3-16 (q) / 4-15 (k, va) for j in 4..15
                    d["qA"] = sbq.tile(
                        [128, 14 * BLOCK], bf16, tag=f"qA{pair}", name=f"qA{pair}"
                    )
                    d["qA_base"] = 3 * BLOCK
                    d["kA"] = sbk.tile(
                        [128, 12 * BLOCK], bf16, tag=f"kA{pair}", name=f"kA{pair}"
                    )
                    d["kA_base"] = 4
                    nc.sync.dma_start(
                        out=d["qA"], in_=qt[0, :, 3 * BLOCK : 17 * BLOCK]
                    )
                    nc.sync.dma_start(
                        out=d["kA"], in_=kt[0, :, 4 * BLOCK : 16 * BLOCK]
                    )
                    d["vA"] = [None, None]
                    d["vA_base"] = 4
                    for s in range(2):
                        tA = sbv.tile(
                            [128, 12 * VW], bf16, tag=f"vA{pair}{s}", name=f"vA{pair}{s}"
                        )
                        nc.sync.dma_start(out=tA, in_=va[0, s, :, 4 * VW : 16 * VW])
                        d["vA"][s] = tA
                else:
                    d["qA"] = sbq.tile(
                        [128, AH], bf16, tag=f"qA{pair}", name=f"qA{pair}"
                    )
                    d["qA_base"] = 0
                    d["kA"] = sbk.tile(
                        [128, 16 * BLOCK], bf16, tag=f"kA{pair}", name=f"kA{pair}"
                    )
                    d["kA_base"] = 0
                    # pair 0's first loads gate the whole pipeline: issue
                    # their descriptors from the three DMA-capable engines
                    # (sync/scalar/gpsimd) in parallel instead of serially
                    # on sync (~650ns of descriptor-gen each).
                    keng = nc.sync if pair else nc.scalar
                    veng = [nc.sync, nc.sync] if pair else [nc.gpsimd, nc.sync]
                    nc.sync.dma_start(out=d["qA"], in_=qt[pair, :, 0:AH])
                    keng.dma_start(out=d["kA"], in_=kt[pair, :, 0 : 16 * BLOCK])
                    d["vA"] = [None, None]
                    d["vA_base"] = 0
                    for s in range(2):
                        tA = sbv.tile(
                            [128, 16 * VW], bf16, tag=f"vA{pair}{s}", name=f"vA{pair}{s}"
                        )
                        veng[s].dma_start(out=tA, in_=va[pair, s, :, 0 : 16 * VW])
                        d["vA"][s] = tA
                d["qB"] = sbq.tile([128, AH], bf16, tag=f"qB{pair}", name=f"qB{pair}")
                d["kB"] = sbk.tile(
                    [128, 16 * BLOCK], bf16, tag=f"kB{pair}", name=f"kB{pair}"
                )
                nc.sync.dma_start(out=d["qB"], in_=qt[pair, :, T - AH : T])
                nc.sync.dma_start(out=d["kB"], in_=kt[pair, :, 16 * BLOCK : T])
                d["vB"] = [None, None]
                for s in range(2):
                    tB = sbv.tile(
                        [128, 16 * VW], bf16, tag=f"vB{pair}{s}", name=f"vB{pair}{s}"
                    )
                    nc.sync.dma_start(out=tB, in_=va[pair, s, :, 16 * VW : NB * VW])
                    d["vB"][s] = tB
                tiles.append(d)

            for pair in range(NPAIR):
                d = tiles[pair]
                pts = {}     # j -> pt tile [128, 768]
                gens = {}    # (s, g) -> psum generation tile
                obts = {}    # (s, t) -> output staging tile

                def qsel(j):
                    if "q0" in d and j <= 3:
                        return d["q0"], 0
                    if j <= 15:
                        return d["qA"], d["qA_base"]
                    return d["qB"], T - AH

                def ksel(j):
                    if "k0" in d and j <= 3:
                        return d["k0"], 0
                    if j <= 15:
                        return d["kA"], d["kA_base"]
                    return d["kB"], 16

                def vsel(j, s):
                    if "v0" in d and j <= 3:
                        return d["v0"][s], 0
                    if j <= 15:
                        return d["vA"][s], d["vA_base"]
                    return d["vB"][s], 16

                def qk(j):
                    """scores^T for key block j of both bh (concurrent)."""
                    sP = psS.tile([128, 1024], f32, tag="spair", name=f"sP{pair}_{j}")
                    # query union = blocks j-1..j+1 clipped; window pos w
                    # covers query block j-1+w at psum cols 128w (+512 bh1)
                    wlo = 1 if j == 0 else 0
                    whi = 2 if j == NB - 1 else 3
                    kta, kbase = ksel(j)
                    qta, qbase = qsel(j)
                    kcol = (j - kbase) * BLOCK
                    qlo = (j - 1 + wlo) * BLOCK - qbase
                    n = (whi - wlo) * BLOCK
                    for s in range(2):
                        p0, p1 = (0, 64) if s == 0 else (64, 128)
                        nc.tensor.matmul(
                            out=sP[:, 512 * s + wlo * BLOCK : 512 * s + wlo * BLOCK + n],
                            lhsT=kta[p0:p1, kcol : kcol + BLOCK],
                            rhs=qta[p0:p1, qlo : qlo + n],
                            start=True,
                            stop=True,
                        )
                    # exp into SBUF bf16 pt tile: [0:384]=bh0, [384:768]=bh1
                    ptj = sbp.tile([128, 768], bf16, tag="pt", name=f"pt{pair}_{j}")
                    s_view = sP.rearrange("p (b w) -> p b w", b=2)[
                        :, :, wlo * BLOCK : whi * BLOCK
                    ]
                    p_view = ptj.rearrange("p (b w) -> p b w", b=2)[
                        :, :, wlo * BLOCK : whi * BLOCK
                    ]
                    if j in DVE_JS:
                        nc.vector.tensor_scalar(
                            out=p_view.bitcast(i16),
                            in0=s_view,
                            scalar1=SCH_A,
                            scalar2=SCH_B,
                            op0=mybir.AluOpType.mult,
                            op1=mybir.AluOpType.add,
                        )
                    else:
                        nc.scalar.activation(
                            out=p_view,
                            in_=s_view,
                            func=mybir.ActivationFunctionType.Exp,
                            bias=bias_tile[:, :],
                            scale=SCALE,
                        )
                    pts[j] = ptj

                def pvq(qb):
                    """One query block's full accumulation: both bh, all 3
                    window keys, back-to-back into the generation bank that
                    holds query blocks (2b, 2b+1) x (bh0, bh1). Generations
                    never overlap, so 2 PSUM banks suffice and the score
                    ring gets depth 3."""
                    b = qb // 2
                    if b not in gens:
                        gens[b] = psG.tile(
                            [128, 512], f32, tag="g", name=f"gen{pair}_{b}"
                        )
                    gt = gens[b]
                    first_of_bank = qb == 2 * b
                    klo, khi = max(0, qb - 1), min(NB - 1, qb + 1)
                    for s in range(2):
                        o = 2 * GSTRIDE * (qb % 2) + GSTRIDE * s
                        for j in range(klo, khi + 1):
                            ptj = pts[j]
                            vaa, vbase = vsel(j, s)
                            w = qb - (j - 1)  # window position 0..2
                            nc.tensor.matmul(
                                out=gt[:, o : o + VW],
                                lhsT=ptj[:, 384 * s + w * BLOCK : 384 * s + w * BLOCK + BLOCK],
                                rhs=vaa[:, (j - vbase) * VW : (j - vbase) * VW + VW],
                                start=(first_of_bank and s == 0 and j == klo),
                                stop=(qb == 2 * b + 1 and s == 1 and j == khi),
                                skip_group_check=True,
                            )

                def close_gen(b):
                    """Copy finished generation bank to staging; DMA per 2.
                    Copy engine alternates so neither ACT nor DVE eats the
                    full copy load on top of its exp share."""
                    gt = gens.pop(b)
                    t = b // 2
                    if t not in obts:
                        obts[t] = sbo.tile(
                            [128, 2 * 4 * VW], bf16, tag="ob", name=f"ob{pair}_{t}"
                        )
                    obt = obts[t]
                    dst = obt.rearrange("p (b w) -> p b w", w=VW)[
                        :, 4 * (b % 2) : 4 * (b % 2) + 4, :
                    ]
                    src = gt[:, 0 : 4 * GSTRIDE].rearrange(
                        "p (b w) -> p b w", w=GSTRIDE
                    )[:, :, 0:VW]
                    if b % 2 == 0:
                        nc.scalar.copy(out=dst, in_=src)
                    else:
                        nc.vector.tensor_copy(out=dst, in_=src)
                    if b % 2 == 1:
                        obts.pop(t)
                        nc.gpsimd.dma_start(out=out[pair, t], in_=obt)

                # PV trails the scores by 3 key blocks (group qb needs
                # pt_{qb+1}), so the PE never waits on a fresh exp.
                for j in range(NB):
                    qk(j)
                    if j >= 3:
                        pvq(j - 3)
                        if (j - 3) % 2 == 1:
                            close_gen((j - 3) // 2)
                pvq(NB - 3)
                close_gen((NB - 3) // 2)
                pvq(NB - 2)
                pvq(NB - 1)
                close_gen((NB - 1) // 2)
                pts.clear()
    nc.compile()
    return nc


def _host_tensors(q, k, v):
    """Build device input arrays from [BH, T, D] fp32 q/k/v.

    qt/kt [BH//2, 128, T]: d-major transposes, bh even on partitions 0-63,
        bh odd on 64-127.
    va [BH//2, 2, 128, NB*65]: per key block j, cols 65j..65j+64 hold
        [v[128j + p, :], 1.0] on partition p.
    """
    qtT = np.ascontiguousarray(q.transpose(0, 2, 1)).astype(_BF16)  # [BH, 64, T]
    ktT = np.ascontiguousarray(k.transpose(0, 2, 1)).astype(_BF16)
    qt = qtT.reshape(BH // 2, 128, T)
    kt = ktT.reshape(BH // 2, 128, T)

    va = np.empty((BH, 128, NB, VW), dtype=_BF16)
    va[:, :, :, :D] = v.reshape(BH, NB, BLOCK, D).transpose(0, 2, 1, 3)
    va[:, :, :, D] = np.float32(1.0)
    va = va.reshape(BH // 2, 2, 128, NB * VW)
    return qt, kt, va


def _in_maps(qt, kt, va):
    maps = []
    for c in range(NCORES):
        s = slice(c * NPAIR, (c + 1) * NPAIR)
        maps.append({"qt": qt[s], "kt": kt[s], "va": va[s]})
    return maps


def _epilogue(outT, q, k, v, mask):
    """outT: [BH//2, 8, 128, 520] bf16 device result -> [N,H,T,D] f32."""
    # 520 = [gen2, qb2, s2, 65]; qb_global = t*4 + gen*2 + qb; bh = 2*pair+s
    o = outT.astype(np.float32).reshape(BH // 2, 8, 128, 2, 2, 2, VW)
    o = o.transpose(0, 5, 1, 3, 4, 2, 6).reshape(BH, NB * BLOCK, VW)
    sums = np.ascontiguousarray(o[:, :, D])      # [BH, T]
    o = np.ascontiguousarray(o[:, :, 0:D])       # [BH, T, D]

    # BOS-token key slot for query blocks >= 2 (blocks 0/1 already have key 0
    # inside their local window, which equals the reference's global slot).
    k0 = k[:, 0, :]
    v0 = v[:, 0, :]
    qs = q[:, 2 * BLOCK :, :]
    pk = np.exp(np.einsum("bqd,bd->bq", qs, k0) * SCALE)
    o[:, 2 * BLOCK :, :] += pk[:, :, None] * v0[:, None, :]
    sums[:, 2 * BLOCK :] += pk

    o /= sums[:, :, None]

    # BOS query row: full attention of query 0 over all T keys.
    mrow = np.repeat(mask[:, 0, 0, :], H, axis=0)  # [BH, T]
    s0 = np.einsum("bd,btd->bt", q[:, 0, :], k) * SCALE + mrow
    s0 -= s0.max(axis=1, keepdims=True)
    p0 = np.exp(s0)
    p0 /= p0.sum(axis=1, keepdims=True)
    o[:, 0, :] = np.einsum("bt,btd->bd", p0, v)

    return o.reshape(N, H, T, D).astype(np.float32)


def kernel(query_layer, key_layer, value_layer, attention_mask):
    from concourse.bass_utils import run_bass_kernel_spmd

    q = np.asarray(query_layer, dtype=np.float32).reshape(BH, T, D)
    k = np.asarray(key_layer, dtype=np.float32).reshape(BH, T, D)
    v = np.asarray(value_layer, dtype=np.float32).reshape(BH, T, D)
    mask = np.asarray(attention_mask, dtype=np.float32)  # [N,1,1,T]

    qt, kt, va = _host_tensors(q, k, v)

    if "nc" not in _CACHE:
        _CACHE["nc"] = _build_bass()
    nc = _CACHE["nc"]

    res = run_bass_kernel_spmd(nc, _in_maps(qt, kt, va), core_ids=list(range(NCORES)))
    outT = np.concatenate([np.asarray(r["out"]) for r in res.results], axis=0)
    return _epilogue(outT, q, k, v, mask)
